# revision 15
# baseline (speedup 1.0000x reference)
"""KMeans criterion kernel for Trainium2 (8 NeuronCores, Bass/Tile).

distances[n,k] = LMBDA * (|e_n|^2 + |c_k|^2 - 2 e_n.c_k)
loss = LMBDA * sum_n max_k distances ;  assignments = argmax_k distances

Strategy: data-parallel over N. Each core handles 16384 rows.
Host pre-transposes embeddings into a [D, N_loc] column-permuted layout so
device DMAs are contiguous and the matmul lhsT (stationary operand) slices
fall out directly. Device computes 2*cross = E^T.T @ (2 C^T) per 128-row
block via 2 fp32 matmuls (contraction D=256 in 2 partition tiles), then on
DVE: t = csq_bcast - psum  [= c_sq - 2*cross], top-8 max + max_index for the
row max and argmax. Per-core partial sums + indices are DMA'd out; host
combines the scalar loss.

Toolchain constraint: this walrus build accepts at most ONE semaphore wait
per instruction. Hence (a) tiny PE "absorber" matmuls soak DMA-completion
waits before real matmuls, (b) a DVE-side copy soaks the csq DMA wait,
(c) total DMA count stays <= 8 so no DMAHW sem lane is ever reused.
"""

import os
import numpy as np

import concourse.bass as bass
import concourse.mybir as mybir
import concourse.tile as tile
from concourse.bass_utils import run_bass_kernel_spmd

from concourse.vector_clock import ScopedClock


def _split_drain_and_barrier(self, tick_clock, wait_clock):
    """Replacement for TileContext._drain_and_barrier: this walrus build
    accepts at most one semaphore wait per instruction, so spread the
    kernel-tail drain's waits over a chain of single-wait drains."""
    nc = self.nc
    drain_inst = nc.sync.drain()
    wait_clock.add_sem_waits(
        drain_inst.ins, ScopedClock({None: tick_clock.global_clock})
    )
    si = drain_inst.ins.sync_info
    if si is not None and len(si.on_wait) > 1:
        waits = list(si.on_wait)
        drain_inst.ins.sync_info = mybir.SyncInfo(on_wait=[waits[0]], on_update=[])
        for w in waits[1:]:
            d2 = nc.sync.drain()
            d2.ins.sync_info = mybir.SyncInfo(on_wait=[w], on_update=[])
    nc.all_engine_barrier()
    assert self.sems is not None
    popped = nc._tile_sem_poison_stack.pop()
    assert popped is self._sem_poison
    nc.clear_and_free_semaphores(list(self.sems.allocated().values()))
    nc.all_engine_barrier()


tile.TileContext._drain_and_barrier = _split_drain_and_barrier

N_CORES = 8
N, K, D = 131072, 512, 256
N_LOC = N // N_CORES        # 16384 rows per core
P = 128                     # partitions
NBLK = N_LOC // P           # 128 row-blocks per core
HEAD = 2048                 # first slice of E^T per half (early PE start)
HEAD_BLKS = HEAD // P
LMBDA = 0.25

F32 = mybir.dt.float32
U32 = mybir.dt.uint32

LAST_RESULTS = None         # stashed for test.py introspection (exec time)
LAST_SPMD_S = None


def _build_nc(repeat=1):
    nc = bass.Bass()
    et = nc.dram_tensor("et", [D, N_LOC], F32, kind="ExternalInput")
    # packed constants: [2*C^T rows 0:128 | 2*C^T rows 128:256 | csq bcast]
    cpack = nc.dram_tensor("cpack", [P, 3 * K], F32, kind="ExternalInput")
    oidx = nc.dram_tensor("out_idx", [N_LOC], U32, kind="ExternalOutput")
    om = nc.dram_tensor("out_m", [P, 1], F32, kind="ExternalOutput")

    with tile.TileContext(nc) as tc:
        with (
            tc.tile_pool(name="const", bufs=1) as constp,
            tc.tile_pool(name="eload", bufs=1) as eload,
            tc.tile_pool(name="tbuf", bufs=4) as tbuf,
            tc.tile_pool(name="acc", bufs=1) as accp,
            tc.tile_pool(name="psum", bufs=6, space="PSUM") as psump,
            tc.tile_pool(name="psd", bufs=1, space="PSUM") as psdp,
        ):
            cp = constp.tile([P, 3 * K], F32, tag="cp")
            nc.sync.dma_start(cp[:], cpack[:])
            ct0 = cp[:, 0:K]
            ct1 = cp[:, K : 2 * K]
            csqt = cp[:, 2 * K : 3 * K]

            e0a = eload.tile([P, HEAD], F32, tag="e0a")
            e1a = eload.tile([P, HEAD], F32, tag="e1a")
            e0b = eload.tile([P, N_LOC - HEAD], F32, tag="e0b")
            e1b = eload.tile([P, N_LOC - HEAD], F32, tag="e1b")
            nc.sync.dma_start(e0a[:], et[0:P, 0:HEAD])
            nc.sync.dma_start(e1a[:], et[P : 2 * P, 0:HEAD])
            nc.sync.dma_start(e0b[:], et[0:P, HEAD:N_LOC])
            nc.sync.dma_start(e1b[:], et[P : 2 * P, HEAD:N_LOC])

            m8_buf = accp.tile([P, NBLK * 8], F32, tag="mbuf")
            idx_buf = accp.tile([P, NBLK * 8], U32, tag="ibuf")
            dve_scratch = accp.tile([P, 1], F32, tag="dsc")

            # single-wait-slot workarounds (see module docstring)
            psd = psdp.tile([1, 1], F32, tag="psd")

            def absorb(ap):
                nc.tensor.matmul(psd[:], ap, ap, start=True, stop=True)

            absorb(ct0[:, 0:1])
            nc.vector.tensor_copy(dve_scratch[:], csqt[:, 0:1])

            def eslice(half, j):
                a, b = (e0a, e0b) if half == 0 else (e1a, e1b)
                if j < HEAD_BLKS:
                    return a[:, j * P : (j + 1) * P]
                return b[:, (j - HEAD_BLKS) * P : (j - HEAD_BLKS + 1) * P]

            for rj in range(repeat * NBLK):
                j = rj % NBLK
                if rj == 0:
                    absorb(e0a[:, 0:1])
                    absorb(e1a[:, 0:1])
                elif rj == HEAD_BLKS:
                    absorb(e0b[:, 0:1])
                    absorb(e1b[:, 0:1])
                ps = psump.tile([P, K], F32, tag="ps")
                nc.tensor.matmul(ps[:], eslice(0, j), ct0, start=True, stop=False)
                nc.tensor.matmul(ps[:], eslice(1, j), ct1, start=False, stop=True)
                t = tbuf.tile([P, K], F32, tag="t")
                m8 = m8_buf[:, j * 8 : (j + 1) * 8]
                # t = csq - 2*cross  (host pre-scaled C^T by 2, so ps = 2*cross)
                nc.vector.tensor_sub(t[:], csqt, ps[:])
                nc.vector.max(m8, t[:])
                nc.vector.max_index(idx_buf[:, j * 8 : (j + 1) * 8], m8, t[:])

            msum = accp.tile([P, 1], F32, tag="msum")
            m8_v = m8_buf[:].rearrange("p (j s) -> p j s", s=8)
            nc.vector.reduce_sum(msum[:], m8_v[:, :, 0], axis=mybir.AxisListType.X)
            nc.sync.dma_start(om[:], msum[:])
            # out_idx mapping: local row n = p*128 + j -> idx_buf[p, j*8]
            oidx_v = oidx.rearrange("(p j) -> p j", p=P)
            ib = idx_buf[:].rearrange("p (j s) -> p j s", s=8)
            nc.sync.dma_start(oidx_v, ib[:, :, 0])
    return nc


_NC = None


def _get_nc():
    global _NC
    if _NC is None:
        _NC = _build_nc(repeat=int(os.environ.get("KERNEL_REPEAT", "1")))
    return _NC


def kernel(embeddings, centroids):
    global LAST_RESULTS, LAST_SPMD_S
    embeddings = np.ascontiguousarray(np.asarray(embeddings, dtype=np.float32))
    centroids = np.ascontiguousarray(np.asarray(centroids, dtype=np.float32))

    csq = (centroids * centroids).sum(axis=1, dtype=np.float32)  # [K]
    ct2 = (2.0 * centroids.T).astype(np.float32)                 # [D, K]
    cpack = np.empty((P, 3 * K), dtype=np.float32)
    cpack[:, 0:K] = ct2[0:P, :]
    cpack[:, K : 2 * K] = ct2[P : 2 * P, :]
    cpack[:, 2 * K : 3 * K] = csq[None, :]

    in_maps = []
    for c in range(N_CORES):
        el = embeddings[c * N_LOC : (c + 1) * N_LOC]
        # ET[d, j*128+p] = el[p*128+j, d]
        et_c = np.ascontiguousarray(
            el.reshape(P, NBLK, D).transpose(2, 1, 0).reshape(D, N_LOC)
        )
        in_maps.append({"et": et_c, "cpack": cpack})

    nc = _get_nc()
    import time as _time
    _t0 = _time.perf_counter()
    res = run_bass_kernel_spmd(
        nc,
        in_maps,
        list(range(N_CORES)),
        trace=bool(os.environ.get("KERNEL_TRACE")),
    )
    LAST_RESULTS = res
    LAST_SPMD_S = _time.perf_counter() - _t0

    assignments = np.concatenate(
        [res.results[c]["out_idx"].astype(np.int32) for c in range(N_CORES)]
    )
    sum_m = sum(
        float(res.results[c]["out_m"].sum(dtype=np.float64)) for c in range(N_CORES)
    )
    sum_esq = float(np.vdot(embeddings, embeddings))
    loss = np.float32(LMBDA * LMBDA * (sum_esq + sum_m))
    return loss, assignments
